# revision 1
# baseline (speedup 1.0000x reference)
# Self-contained kernel for nn_Convolution_22917945491528 (e3nn-style GNN conv).
# Strategy: full device offload on 8 TRN2 NeuronCores (edge-parallel, dst-window
# bucketed). Per core: indirect-gather of source-node features, radial MLP,
# CG tensor product in bf16 spread over DVE/ACT/Pool, one-hot selector matmuls
# accumulating per-128-node-window sums in PSUM, lin2 via PE transposes, output
# node-sharded. Host does lin1/self-connection/bucketing/final combine.
# Falls back to a pure-numpy path if the device is unavailable.
import numpy as np

N_NODES, N_EDGES = 10000, 160000
MUL0, MUL1 = 64, 32
P = 128
N_CORES = 8
WPC = 10          # 128-node windows per core
CC = 18           # chunk budget (x128 edges) per window
NCHUNKS = WPC * CC
NECS = NCHUNKS * P          # edge slots per core
NPC = WPC * P               # nodes per core
NTAB = N_CORES * NPC        # 10240 table rows

SQ3, SQ5 = float(np.sqrt(3.0)), float(np.sqrt(5.0))
W112_TERMS = [
    (0, 0, 2, +0.18257419), (0, 0, 4, +0.31622777), (0, 1, 1, -0.31622777),
    (0, 2, 0, -0.31622777), (1, 0, 1, -0.31622777), (1, 1, 2, -0.36514837),
    (1, 2, 3, -0.31622777), (2, 0, 0, -0.31622777), (2, 1, 3, -0.31622777),
    (2, 2, 2, +0.18257419), (2, 2, 4, -0.31622777),
]
W121_TERMS = [
    (0, 0, 2, +0.31622777), (0, 1, 1, +0.31622777), (0, 2, 0, -0.18257419),
    (0, 4, 0, -0.31622777), (1, 1, 0, +0.31622777), (1, 2, 1, +0.36514837),
    (1, 3, 2, +0.31622777), (2, 0, 0, +0.31622777), (2, 2, 2, -0.18257419),
    (2, 3, 1, +0.31622777), (2, 4, 2, +0.31622777),
]
_x, _w = np.polynomial.hermite_e.hermegauss(128)
_s = _x / (1 + np.exp(-_x))
SILU_C = float(1.0 / np.sqrt((_w * _s ** 2).sum() / _w.sum()))

# block->reference output column map: ref col r <- block col _COLPERM[r]
_COLPERM = np.empty(320, np.int64)
_COLPERM[:64] = np.arange(64)
for _v in range(32):
    for _i in range(3):
        _COLPERM[64 + _v * 3 + _i] = 64 + _i * 32 + _v
    for _i in range(5):
        _COLPERM[160 + _v * 5 + _i] = 160 + _i * 32 + _v

LAST_EXEC_NS = None
_DEV = {}


# ---------------------------------------------------------------------------
# BIR post-pass: this walrus build allows at most ONE sem wait per
# instruction; hoist excess waits onto same-engine NoOp carriers.
def _split_waits(nc, mybir, limit=1):
    def engine_api(engine_type):
        s = str(engine_type)
        if "SP" in s:
            return nc.sync
        if "Activation" in s:
            return nc.scalar
        if "DVE" in s:
            return nc.vector
        if "PE" in s:
            return nc.tensor
        if "Pool" in s:
            return nc.gpsimd
        raise ValueError(s)

    for f in nc.m.functions:
        for b in f.blocks:
            out = []
            for ins in list(b.instructions):
                si = getattr(ins, "sync_info", None)
                ow = list(si.on_wait) if (si and si.on_wait) else []
                if len(ow) > limit:
                    excess, keep = ow[:-limit], ow[-limit:]
                    for i in range(0, len(excess), limit):
                        chunk = excess[i:i + limit]
                        bi = engine_api(ins.engine).nop(nofuse=True)
                        nop_ins = bi.ins
                        found = False
                        for f2 in nc.m.functions:
                            for b2 in reversed(list(f2.blocks)):
                                bl = list(b2.instructions)
                                if bl and bl[-1] is nop_ins:
                                    b2.instructions.pop()
                                    found = True
                                    break
                            if found:
                                break
                        if not found:
                            for f2 in nc.m.functions:
                                for b2 in f2.blocks:
                                    if nop_ins in b2.instructions:
                                        b2.instructions.remove(nop_ins)
                        if nop_ins.sync_info is None:
                            nop_ins.sync_info = mybir.SyncInfo(on_wait=[], on_update=[])
                        nop_ins.sync_info.on_wait.extend(chunk)
                        out.append(nop_ins)
                    del si.on_wait[:]
                    si.on_wait.extend(keep)
                out.append(ins)
            del b.instructions[:]
            for i in out:
                b.instructions.append(i)


def _build_conv():
    import concourse.bass as bass
    import concourse.mybir as mybir
    from concourse.tile import TileContext
    from concourse.masks import make_identity

    F32 = mybir.dt.float32
    dtype = mybir.dt.bfloat16
    AF = mybir.ActivationFunctionType
    ALU = mybir.AluOpType
    wpc, cc, ntab = WPC, CC, NTAB

    nchunks = wpc * cc
    necs = nchunks * P
    EW = cc * P

    nc = bass.Bass()
    ytab = nc.dram_tensor("ytab", [ntab, 160], dtype, kind="ExternalInput")
    eleT = nc.dram_tensor("eleT", [8, necs], dtype, kind="ExternalInput")
    eaT_d = nc.dram_tensor("eaT", [9, necs], dtype, kind="ExternalInput")
    srcT_d = nc.dram_tensor("srcT", [P, nchunks], mybir.dt.int32, kind="ExternalInput")
    dstT_d = nc.dram_tensor("dstT", [P, nchunks], dtype, kind="ExternalInput")
    iota_d = nc.dram_tensor("iota", [P, P], dtype, kind="ExternalInput")
    fw0_d = nc.dram_tensor("fw0", [8, 64], dtype, kind="ExternalInput")
    fw1_d = nc.dram_tensor("fw1", [64, 320], dtype, kind="ExternalInput")
    lw0_d = nc.dram_tensor("lw0", [96, 64], dtype, kind="ExternalInput")
    lw1_d = nc.dram_tensor("lw1", [128, 32], dtype, kind="ExternalInput")
    lw2_d = nc.dram_tensor("lw2", [96, 32], dtype, kind="ExternalInput")
    out_d = nc.dram_tensor("out", [wpc * P, 320], dtype, kind="ExternalOutput")

    with TileContext(nc) as tc:
        with (
            tc.tile_pool(name="const", bufs=1) as cpool,
            tc.tile_pool(name="sb", bufs=2) as pool,
            tc.tile_pool(name="big", bufs=1) as bpool,
            tc.tile_pool(name="psA", bufs=1, space="PSUM") as psA,
            tc.tile_pool(name="psW", bufs=2, space="PSUM") as psW,
            tc.tile_pool(name="psT", bufs=3, space="PSUM") as psT,
        ):
            iota_t = cpool.tile([P, P], dtype, tag="iota")
            nc.sync.dma_start(out=iota_t[:], in_=iota_d[:])
            ident = cpool.tile([P, P], dtype, tag="ident")
            make_identity(nc, ident[:])
            fw0_t = cpool.tile([8, 64], dtype, tag="fw0")
            nc.sync.dma_start(out=fw0_t[:], in_=fw0_d[:])
            fw1_t = cpool.tile([64, 320], dtype, tag="fw1")
            nc.sync.dma_start(out=fw1_t[:], in_=fw1_d[:])
            lw0a = cpool.tile([64, 64], dtype, tag="lw0a")
            nc.sync.dma_start(out=lw0a[:], in_=lw0_d[0:64, :])
            lw0b = cpool.tile([32, 64], dtype, tag="lw0b")
            nc.sync.dma_start(out=lw0b[:], in_=lw0_d[64:96, :])
            lw1a = cpool.tile([64, 32], dtype, tag="lw1a")
            nc.sync.dma_start(out=lw1a[:], in_=lw1_d[0:64, :])
            lw1b = cpool.tile([32, 32], dtype, tag="lw1b")
            nc.sync.dma_start(out=lw1b[:], in_=lw1_d[64:96, :])
            lw1c = cpool.tile([32, 32], dtype, tag="lw1c")
            nc.sync.dma_start(out=lw1c[:], in_=lw1_d[96:128, :])
            lw2a = cpool.tile([64, 32], dtype, tag="lw2a")
            nc.sync.dma_start(out=lw2a[:], in_=lw2_d[0:64, :])
            lw2b = cpool.tile([32, 32], dtype, tag="lw2b")
            nc.sync.dma_start(out=lw2b[:], in_=lw2_d[64:96, :])
            srcT = cpool.tile([P, nchunks], mybir.dt.int32, tag="srcT")
            nc.sync.dma_start(out=srcT[:], in_=srcT_d[:])
            dstT = cpool.tile([P, nchunks], dtype, tag="dstT")
            nc.sync.dma_start(out=dstT[:], in_=dstT_d[:])

            def TT(out, i0, i1):
                nc.vector.tensor_tensor(out=out, in0=i0, in1=i1, op=ALU.mult)

            for w in range(wpc):
                e0 = w * EW

                hT = bpool.tile([64, EW], dtype, tag="hT")
                for g in range(0, EW, 512):
                    sz = min(512, EW - g)
                    ele_sb = pool.tile([8, 512], dtype, tag="elesb")
                    nc.sync.dma_start(out=ele_sb[:, :sz], in_=eleT[:, e0 + g:e0 + g + sz])
                    h_ps = psW.tile([64, 512], F32, tag="hps")
                    nc.tensor.matmul(h_ps[:, :sz], lhsT=fw0_t[:], rhs=ele_sb[:, :sz],
                                     start=True, stop=True)
                    nc.scalar.activation(hT[:, g:g + sz], h_ps[:, :sz], AF.Silu)
                w0t = bpool.tile([64, EW], dtype, tag="w0t")
                w2t = bpool.tile([64, EW], dtype, tag="w2t")
                w5t = bpool.tile([64, EW], dtype, tag="w5t")
                w3t = bpool.tile([32, EW], dtype, tag="w3t")
                w1t = bpool.tile([32, EW], dtype, tag="w1t")
                w6t = bpool.tile([32, EW], dtype, tag="w6t")
                w4t = bpool.tile([32, EW], dtype, tag="w4t")
                wplan = [((0, 128), [(w0t, 0, 64), (w2t, 64, 128)]),
                         ((128, 256), [(w5t, 0, 64), (w3t, 64, 96), (w1t, 96, 128)]),
                         ((256, 320), [(w6t, 0, 32), (w4t, 32, 64)])]
                for ((cb, ce), dsts) in wplan:
                    cwd = ce - cb
                    for g in range(0, EW, 512):
                        sz = min(512, EW - g)
                        w_ps = psW.tile([P, 512], F32, tag="wps")
                        nc.tensor.matmul(w_ps[:cwd, :sz], lhsT=fw1_t[:, cb:ce],
                                         rhs=hT[:, g:g + sz], start=True, stop=True)
                        for (dt_, r0, r1) in dsts:
                            nc.scalar.activation(dt_[:r1 - r0, g:g + sz],
                                                 w_ps[r0:r1, :sz], AF.Copy)

                x0T = bpool.tile([64, EW], dtype, tag="x0T")
                x1a = bpool.tile([32, EW], dtype, tag="x1a")
                x1b = bpool.tile([32, EW], dtype, tag="x1b")
                x1c = bpool.tile([32, EW], dtype, tag="x1c")
                x1T = [x1a, x1b, x1c]
                for k in range(cc):
                    c = w * cc + k
                    xs = pool.tile([P, 160], dtype, tag="xs")
                    nc.gpsimd.indirect_dma_start(
                        out=xs[:], out_offset=None, in_=ytab[:],
                        in_offset=bass.IndirectOffsetOnAxis(ap=srcT[:, c:c + 1], axis=0))
                    tp1 = psT.tile([P, P], dtype, tag="pst")
                    nc.tensor.transpose(out=tp1[:], in_=xs[:, 0:128], identity=ident[:])
                    nc.scalar.activation(x0T[:, k * P:(k + 1) * P], tp1[0:64, :], AF.Copy)
                    nc.scalar.activation(x1a[:, k * P:(k + 1) * P], tp1[64:96, :], AF.Copy)
                    nc.scalar.activation(x1b[:, k * P:(k + 1) * P], tp1[96:128, :], AF.Copy)
                    tp2 = psT.tile([P, P], dtype, tag="pst")
                    nc.tensor.transpose(out=tp2[:32, :], in_=xs[:, 128:160], identity=ident[:])
                    nc.scalar.activation(x1c[:, k * P:(k + 1) * P], tp2[0:32, :], AF.Copy)

                ebs = []
                for row in range(9):
                    ebt = bpool.tile([P, EW], dtype, tag=f"ebc{row}")
                    nc.sync.dma_start(
                        out=ebt[:], in_=eaT_d[row:row + 1, e0:e0 + EW].to_broadcast([P, EW]))
                    ebs.append(ebt)
                e0b, e1b, e2b = ebs[0], ebs[1:4], ebs[4:9]

                t0 = bpool.tile([64, EW], dtype, tag="t0")
                TT(t0[:], x0T[:], w0t[:])
                t2 = bpool.tile([64, EW], dtype, tag="t2")
                TT(t2[:], x0T[:], w2t[:])
                t5 = bpool.tile([64, EW], dtype, tag="t5")
                TT(t5[:], x0T[:], w5t[:])
                r4 = []
                r6 = []
                for i in range(3):
                    r4t = bpool.tile([32, EW], dtype, tag=f"r4_{i}")
                    TT(r4t[:], x1T[i][:], w4t[:])
                    r4.append(r4t)
                    r6t = bpool.tile([32, EW], dtype, tag=f"r6_{i}")
                    TT(r6t[:], x1T[i][:], w6t[:])
                    r6.append(r6t)

                k0t = bpool.tile([64, EW], dtype, tag="k0t")
                TT(k0t[:], t0[:], e0b[0:64, :])
                k1t = bpool.tile([32, EW], dtype, tag="k1t")
                tmq = bpool.tile([32, EW], dtype, tag="tmq")
                TT(k1t[:], x1T[0][:], e1b[0][0:32, :])
                TT(tmq[:], x1T[1][:], e1b[1][0:32, :])
                nc.vector.tensor_tensor(out=k1t[:], in0=k1t[:], in1=tmq[:], op=ALU.add)
                TT(tmq[:], x1T[2][:], e1b[2][0:32, :])
                nc.vector.tensor_tensor(out=k1t[:], in0=k1t[:], in1=tmq[:], op=ALU.add)
                TT(k1t[:], k1t[:], w1t[:])

                sc = bpool.tile([32, EW], dtype, tag="sc")
                tm = bpool.tile([32, EW], dtype, tag="tm")

                agg_ps = psA.tile([P, 320], F32, tag="agg")
                T0 = bpool.tile([P, EW], dtype, tag="T0")
                T1 = bpool.tile([P, EW], dtype, tag="T1")
                T2 = bpool.tile([64, EW], dtype, tag="T2")

                for g in range(0, EW, 512):
                    sz = min(512, EW - g)
                    o_ps = psW.tile([P, 512], F32, tag="wps")
                    nc.tensor.matmul(o_ps[:64, :sz], lhsT=lw0a[:], rhs=k0t[:, g:g + sz],
                                     start=True, stop=False)
                    nc.tensor.matmul(o_ps[:64, :sz], lhsT=lw0b[:], rhs=k1t[:, g:g + sz],
                                     start=False, stop=True)
                    nc.scalar.activation(T0[0:64, g:g + sz], o_ps[:64, :sz], AF.Copy)

                k2t = bpool.tile([64, EW], dtype, tag="k2t")
                k3t = bpool.tile([32, EW], dtype, tag="k3t")
                k4t = bpool.tile([32, EW], dtype, tag="k4t")
                o1dst = [(T0, 64), (T0, 96), (T1, 0)]
                for i in range(3):
                    TT(k2t[:], t2[:], e1b[i][0:64, :])
                    TT(k3t[:], x1T[i][:], w3t[:])
                    TT(k3t[:], k3t[:], e0b[0:32, :])
                    terms = [(ii, j, cf) for (ii, j, kk, cf) in W121_TERMS if kk == i]
                    for ti, (ii, j, cf) in enumerate(terms):
                        nc.vector.tensor_scalar(out=sc[:], in0=e2b[j][0:32, :],
                                                scalar1=float(cf * SQ3), scalar2=None,
                                                op0=ALU.mult)
                        tgt = k4t[:] if ti == 0 else tm[:]
                        TT(tgt, r4[ii][:], sc[:])
                        if ti:
                            nc.vector.tensor_tensor(out=k4t[:], in0=k4t[:], in1=tm[:], op=ALU.add)
                    Tt, ro = o1dst[i]
                    for g in range(0, EW, 512):
                        sz = min(512, EW - g)
                        o_ps = psW.tile([P, 512], F32, tag="wps")
                        nc.tensor.matmul(o_ps[:32, :sz], lhsT=lw1a[:], rhs=k2t[:, g:g + sz],
                                         start=True, stop=False)
                        nc.tensor.matmul(o_ps[:32, :sz], lhsT=lw1b[:], rhs=k3t[:, g:g + sz],
                                         start=False, stop=False)
                        nc.tensor.matmul(o_ps[:32, :sz], lhsT=lw1c[:], rhs=k4t[:, g:g + sz],
                                         start=False, stop=True)
                        nc.scalar.activation(Tt[ro:ro + 32, g:g + sz], o_ps[:32, :sz], AF.Copy)

                k5t = bpool.tile([64, EW], dtype, tag="k5t")
                k6t = bpool.tile([32, EW], dtype, tag="k6t")
                o2dst = [(T1, 32), (T1, 64), (T1, 96), (T2, 0), (T2, 32)]
                for i in range(5):
                    TT(k5t[:], t5[:], e2b[i][0:64, :])
                    terms = [(ii, j, cf) for (ii, j, kk, cf) in W112_TERMS if kk == i]
                    for ti, (ii, j, cf) in enumerate(terms):
                        nc.vector.tensor_scalar(out=sc[:], in0=e1b[j][0:32, :],
                                                scalar1=float(cf * SQ5), scalar2=None,
                                                op0=ALU.mult)
                        tgt = k6t[:] if ti == 0 else tm[:]
                        TT(tgt, r6[ii][:], sc[:])
                        if ti:
                            nc.vector.tensor_tensor(out=k6t[:], in0=k6t[:], in1=tm[:], op=ALU.add)
                    Tt, ro = o2dst[i]
                    for g in range(0, EW, 512):
                        sz = min(512, EW - g)
                        o_ps = psW.tile([P, 512], F32, tag="wps")
                        nc.tensor.matmul(o_ps[:32, :sz], lhsT=lw2a[:], rhs=k5t[:, g:g + sz],
                                         start=True, stop=False)
                        nc.tensor.matmul(o_ps[:32, :sz], lhsT=lw2b[:], rhs=k6t[:, g:g + sz],
                                         start=False, stop=True)
                        nc.scalar.activation(Tt[ro:ro + 32, g:g + sz], o_ps[:32, :sz], AF.Copy)

                for k in range(cc):
                    c = w * cc + k
                    g = k * P
                    rhs_t = pool.tile([P, 320], dtype, tag="rhs")
                    tpa = psT.tile([P, P], dtype, tag="pst")
                    nc.tensor.transpose(out=tpa[:], in_=T0[:, g:g + P], identity=ident[:])
                    nc.scalar.activation(rhs_t[:, 0:128], tpa[:], AF.Copy)
                    tpb = psT.tile([P, P], dtype, tag="pst")
                    nc.tensor.transpose(out=tpb[:], in_=T1[:, g:g + P], identity=ident[:])
                    nc.scalar.activation(rhs_t[:, 128:256], tpb[:], AF.Copy)
                    tpc = psT.tile([P, P], dtype, tag="pst")
                    nc.tensor.transpose(out=tpc[:, :64], in_=T2[:, g:g + P],
                                        identity=ident[:64, :64])
                    nc.scalar.activation(rhs_t[:, 256:320], tpc[:, :64], AF.Copy)
                    oh = pool.tile([P, P], dtype, tag="oh")
                    nc.vector.tensor_tensor(out=oh[:], in0=dstT[:, c:c + 1].to_broadcast([P, P]),
                                            in1=iota_t[:], op=ALU.is_equal)
                    nc.tensor.matmul(agg_ps[:], lhsT=oh[:], rhs=rhs_t[:],
                                     start=(k == 0), stop=(k == cc - 1))
                out_sb = pool.tile([P, 320], dtype, tag="outsb")
                nc.scalar.activation(out_sb[:], agg_ps[:], AF.Copy)
                nc.sync.dma_start(out=out_d[w * P:(w + 1) * P, :], in_=out_sb[:])
    import concourse.mybir as mybir2
    _split_waits(nc, mybir2, limit=1)
    return nc


def _init_device():
    """Build + compile + warm-run once. Returns True on success."""
    if 'ok' in _DEV:
        return _DEV['ok']
    try:
        import ml_dtypes
        from concourse.bass_utils import run_bass_kernel_spmd
        nc = _build_conv()
        _DEV['nc'] = nc
        _DEV['run'] = run_bass_kernel_spmd
        _DEV['bf'] = ml_dtypes.bfloat16
        bf = ml_dtypes.bfloat16
        iota = np.tile(np.arange(P, dtype=np.float32), (P, 1)).astype(bf)
        _DEV['iota'] = iota
        zim = dict(
            ytab=np.zeros((NTAB, 160), bf), eleT=np.zeros((8, NECS), bf),
            eaT=np.zeros((9, NECS), bf),
            srcT=np.zeros((P, NCHUNKS), np.int32),
            dstT=np.full((P, NCHUNKS), 200.0, bf), iota=iota,
            fw0=np.zeros((8, 64), bf), fw1=np.zeros((64, 320), bf),
            lw0=np.zeros((96, 64), bf), lw1=np.zeros((128, 32), bf),
            lw2=np.zeros((96, 32), bf))
        run_bass_kernel_spmd(nc, [zim] * N_CORES, core_ids=list(range(N_CORES)))
        _DEV['ok'] = True
    except Exception as e:
        import sys, traceback
        print("device init failed, will use host fallback:", repr(e)[:200], file=sys.stderr)
        traceback.print_exc()
        _DEV['ok'] = False
    return _DEV['ok']


def kernel(node_input, node_attr, edge_src, edge_dst, edge_attr,
           edge_length_embedded, sc_w0, sc_w1, lin1_w0, lin1_w1,
           fc_w0, fc_w1, lin2_w0, lin2_w1, lin2_w2):
    f32 = np.float32
    x = np.asarray(node_input, f32)
    a = np.asarray(node_attr, f32)
    src = np.asarray(edge_src, np.int64)
    dst = np.asarray(edge_dst, np.int64)
    ea = np.asarray(edge_attr, f32)
    ele = np.asarray(edge_length_embedded, f32)
    N, E = N_NODES, N_EDGES
    c_s = f32(np.sin(np.pi / 8))
    c_x = f32(np.cos(np.pi / 8))

    xa = x * a
    x0 = xa[:, :MUL0]
    x1 = xa[:, MUL0:].reshape(N, MUL1, 3)

    # self connection (c_s folded)
    s0 = x0 @ (sc_w0 * (c_s / 8.0)).astype(f32)
    s1 = np.einsum('nui,uv->nvi', x1, (sc_w1 * (c_s / np.sqrt(32.0))).astype(f32))

    # lin1 -> y  [N,160]
    y0 = x0 @ (lin1_w0 / 8.0).astype(f32)
    y1 = np.einsum('nui,uv->nvi', x1, (lin1_w1 / np.sqrt(32.0)).astype(f32))
    y = np.concatenate([y0, y1.transpose(0, 2, 1).reshape(N, 96)], 1)

    devout = None
    win = (dst // P).astype(np.int64)
    counts = np.bincount(win, minlength=N_CORES * WPC)
    if counts.max() <= CC * P and _init_device():
        try:
            bf = _DEV['bf']
            fw0s = (fc_w0 / np.sqrt(8.0)).astype(bf)
            fw1s = (fc_w1 * (SILU_C / 8.0)).astype(bf)
            lw0s = (lin2_w0 * (c_x / (4.0 * np.sqrt(96.0)))).astype(f32)
            lw0s[64:96] /= SQ3
            lw0s = lw0s.astype(bf)
            lw1s = (lin2_w1 * (c_x / (4.0 * np.sqrt(128.0)))).astype(bf)
            lw2s = (lin2_w2 * (1.0 / (4.0 * np.sqrt(96.0)))).astype(bf)
            ytab_np = np.zeros((NTAB, 160), bf)
            ytab_np[:N] = y.astype(bf)

            order = np.argsort(win, kind='stable')
            win_s = win[order]
            starts = np.zeros(N_CORES * WPC, np.int64)
            starts[1:] = np.cumsum(counts)[:-1]
            pos = win_s * (CC * P) + (np.arange(E) - starts[win_s])
            EPAD = N_CORES * WPC * CC * P
            ele_p = np.zeros((EPAD, 8), f32)
            ele_p[pos] = ele[order]
            ea_p = np.zeros((EPAD, 9), f32)
            ea_p[pos] = ea[order]
            src_p = np.zeros(EPAD, np.int32)
            src_p[pos] = src[order]
            dstl_p = np.full(EPAD, 200.0, f32)
            dstl_p[pos] = (dst - win * P)[order]

            ele_b = ele_p.astype(bf)
            ea_b = ea_p.astype(bf)
            # chunk-column layouts [P, NCHUNKS] per core
            src_r = src_p.reshape(N_CORES, NCHUNKS, P)
            dstl_b = dstl_p.astype(bf).reshape(N_CORES, NCHUNKS, P)
            in_maps = []
            for cidx in range(N_CORES):
                sl = slice(cidx * NECS, (cidx + 1) * NECS)
                in_maps.append(dict(
                    ytab=ytab_np,
                    eleT=np.ascontiguousarray(ele_b[sl].T),
                    eaT=np.ascontiguousarray(ea_b[sl].T),
                    srcT=np.ascontiguousarray(src_r[cidx].T),
                    dstT=np.ascontiguousarray(dstl_b[cidx].T),
                    iota=_DEV['iota'], fw0=fw0s, fw1=fw1s,
                    lw0=lw0s, lw1=lw1s, lw2=lw2s))
            res = _DEV['run'](_DEV['nc'], in_maps, core_ids=list(range(N_CORES)))
            devb = np.concatenate(
                [res.results[cidx]['out'].astype(f32) for cidx in range(N_CORES)], 0)[:N]
            devout = devb[:, _COLPERM]
        except Exception as e:
            import sys, traceback
            print("device run failed, host fallback:", repr(e)[:200], file=sys.stderr)
            traceback.print_exc()
            devout = None

    if devout is None:
        devout = _host_edges(y, src, dst, ea, ele, fc_w0, fc_w1,
                             lin2_w0, lin2_w1, lin2_w2, c_x)

    out = np.empty((N, 320), f32)
    out[:, :64] = s0 + devout[:, :64] * a
    out[:, 64:160] = s1.reshape(N, 96) + devout[:, 64:160] * a
    out[:, 160:320] = devout[:, 160:320] * a
    return out


def _host_edges(y, src, dst, ea, ele, fc_w0, fc_w1, lin2_w0, lin2_w1, lin2_w2, c_x):
    """Numpy fallback: edge pipeline + aggregation + lin2 (pre node_attr)."""
    f32 = np.float32
    N, E = N_NODES, N_EDGES
    # sort by dst first so no big permutation later
    order = np.argsort(dst, kind='stable')
    srcs, dsts = src[order], dst[order]
    pre = ele[order] @ (fc_w0 / np.sqrt(8.0)).astype(f32)
    h = pre / (1.0 + np.exp(-pre))
    w = h @ (fc_w1 * (SILU_C / 8.0)).astype(f32)
    eas = ea[order]
    xs = y[srcs]
    xs0 = xs[:, :64]
    xs1 = xs[:, 64:].reshape(E, 3, 32).transpose(0, 2, 1)  # y table is i-major
    e0 = eas[:, 0:1]
    e1 = eas[:, 1:4]
    e2 = eas[:, 4:9]

    feat = np.empty((E, 960), f32)
    t0 = xs0 * w[:, 0:64]
    t2 = xs0 * w[:, 64:128]
    t5 = xs0 * w[:, 128:192]
    feat[:, 0:64] = t0 * e0
    feat[:, 64:96] = (np.einsum('eui,ei->eu', xs1, e1) / SQ3) * w[:, 224:256]
    feat[:, 96:288] = (t2[:, :, None] * e1[:, None, :]).reshape(E, 192)
    feat[:, 288:384] = (xs1 * w[:, 192:224][:, :, None] * e0[:, :, None]).reshape(E, 96)
    k4 = np.zeros((E, 32, 3), f32)
    for (i, j, k, cf) in W121_TERMS:
        k4[:, :, k] += (SQ3 * cf) * xs1[:, :, i] * e2[:, j:j + 1]
    feat[:, 384:480] = (k4 * w[:, 288:320][:, :, None]).reshape(E, 96)
    feat[:, 480:800] = (t5[:, :, None] * e2[:, None, :]).reshape(E, 320)
    k6 = np.zeros((E, 32, 5), f32)
    for (i, j, k, cf) in W112_TERMS:
        k6[:, :, k] += (SQ5 * cf) * xs1[:, :, i] * e1[:, j:j + 1]
    feat[:, 800:960] = (k6 * w[:, 256:288][:, :, None]).reshape(E, 160)

    bounds = np.searchsorted(dsts, np.arange(N))
    agg = np.add.reduceat(
        np.concatenate([feat, np.zeros((1, 960), f32)], 0),
        np.minimum(bounds, E), axis=0)[:N]
    agg[np.bincount(dsts, minlength=N) == 0] = 0

    m0 = agg[:, :96]
    m1 = agg[:, 96:480].reshape(N, 128, 3)
    m2 = agg[:, 480:960].reshape(N, 96, 5)
    o0 = m0 @ (lin2_w0 * (c_x / (4 * np.sqrt(96.0)))).astype(f32)
    o1 = np.einsum('nui,uv->nvi', m1, (lin2_w1 * (c_x / (4 * np.sqrt(128.0)))).astype(f32))
    o2 = np.einsum('nui,uv->nvi', m2, (lin2_w2 * (1.0 / (4 * np.sqrt(96.0)))).astype(f32))
    out = np.empty((N, 320), f32)
    out[:, :64] = o0
    out[:, 64:160] = o1.reshape(N, 96)
    out[:, 160:320] = o2.reshape(N, 160)
    return out


_init_device()

